# revision 1
# baseline (speedup 1.0000x reference)
"""Complex LSTM cell (CLSTMCell) Trainium2 kernel.

Full inputs in, full outputs out. Data-parallel over batch: B=4096 rows
sharded 512/core across 8 NeuronCores; the 8 complex weight matrices are
replicated (host pre-packed into a matmul-friendly layout).

Math: with X1=[xr|hr], X2=[xi|hi] ([B,2048]) and W1=[Ur;Wr], W2=[Ui;Wi]
([2048,4096]), the complex gate projection is computed via Karatsuba:
  P1 = X1@W1, P2 = X2@W2, P3 = (X1+X2)@(W1+W2)
  Zr = P1 - P2 (+ br),  Zi = P3 - P1 - P2 (+ bi)
i.e. 3 real matmuls instead of 4 (25% FLOP cut). Matmuls run in bf16
(fp32 PSUM accumulation); the elementwise gate epilogue runs in fp32.

Weight columns are interleaved as c = oblk*512 + gate*128 + (o % 128)
so each N=512 matmul block contains all 4 gates for one 128-wide o
slice, letting the cell update complete per-block with no cross-block
buffering.
"""

import sys

for _p in ("/opt/trn_rl_repo",):
    if _p not in sys.path:
        sys.path.insert(0, _p)

import numpy as np
import ml_dtypes

import concourse.bass as bass
import concourse.mybir as mybir
from concourse.bass_utils import run_bass_kernel_spmd
from concourse.tile import TileContext, add_dep_helper

F32 = mybir.dt.float32
BF16 = mybir.dt.bfloat16
AFT = mybir.ActivationFunctionType

B = 4096
IN = 1024
H = 1024
G = 4
NCORES = 8
BL = B // NCORES          # 512 batch rows per core
MT = BL // 128            # 4 m-tiles per core
K = 2 * IN                # 2048 contraction dim (x|h concat)
KT = K // 128             # 16 k-tiles
OB = H // 128             # 8 o-blocks
NW = G * 128              # 512 matmul N (all gates for one o-block)


def _split_multiwait_json(raw: bytes) -> bytes:
    """The walrus build in this container accepts at most one sem wait
    per instruction; Tile's scheduler packs several. Split the extras
    into preceding wait-only EventSemaphore instructions on the same
    engine (same semantics: the sequencer blocks on each in order)."""
    import orjson

    m = orjson.loads(raw)
    ctr = 0
    for fn in m["functions"]:
        for bb in fn["blocks"]:
            out = []
            for ins in bb["instructions"]:
                si = ins.get("sync_info")
                waits = si.get("on_wait") if si else None
                if waits and len(waits) > 1:
                    for w in waits[:-1]:
                        ctr += 1
                        nop = {
                            "engine": ins["engine"],
                            "ins": [],
                            "outs": [],
                            "name": f"{ins['name']}_sw{ctr}",
                            "opcode": "EventSemaphore",
                            "sync_info": {"on_update": [], "on_wait": [w]},
                        }
                        if "debug" in ins:
                            nop["debug"] = ins["debug"]
                        out.append(nop)
                    si["on_wait"] = [waits[-1]]
                out.append(ins)
            bb["instructions"] = out
    return orjson.dumps(m)


def _build_program(repeat=1, timing=False):
    # timing=True builds a NEFF with Internal (unshipped) I/O and the main
    # loop emitted `repeat` times — used only to measure per-step device
    # time without the host<->device transfer cost dominating.
    nc = bass.Bass()

    kin = "Internal" if timing else "ExternalInput"
    kout = "Internal" if timing else "ExternalOutput"
    x1tp = nc.dram_tensor("x1tp", [KT, 128, BL], BF16, kind=kin)
    x2tp = nc.dram_tensor("x2tp", [KT, 128, BL], BF16, kind=kin)
    cx = nc.dram_tensor("cx", [BL, 2 * H], BF16, kind=kin)
    wpk = nc.dram_tensor("wpk", [3, OB, 128, KT, NW], BF16, kind=kin)
    bbc = nc.dram_tensor("bbc", [2, 128, G * H], BF16, kind="ExternalInput")
    h_out = nc.dram_tensor("h_out", [BL, 2 * H], F32, kind=kout)
    c_out = nc.dram_tensor("c_out", [BL, 2 * H], F32, kind=kout)
    sink = (
        nc.dram_tensor("sink", [1, 4], F32, kind="ExternalOutput")
        if timing
        else None
    )

    with TileContext(nc) as tc:
        with (
            tc.tile_pool(name="const", bufs=1) as constp,
            tc.tile_pool(name="cres", bufs=1) as cresp,
            tc.tile_pool(name="xt", bufs=1) as xtp,
            tc.tile_pool(name="w", bufs=3) as wp,
            tc.tile_pool(name="pp", bufs=1) as pp,
            tc.tile_pool(name="ep", bufs=2) as epp,
            tc.tile_pool(name="prod", bufs=2) as prodp,
            tc.tile_pool(name="ps_mm", bufs=6, space="PSUM") as psmm,
        ):
            # bias rows land as two 8KB DMAs and are partition-broadcast
            # in place by GPSIMD — keeps the SP HWDGE ring free for the
            # first weight slab at kernel start.
            bias_r = constp.tile([128, G * H], BF16, tag="bias_r", name="bias_r")
            bias_i = constp.tile([128, G * H], BF16, tag="bias_i", name="bias_i")
            bias_dma_holder = []

            ctile = []

            # X1.T / X2.T k-tiles in bf16 via cast-DMA + PE transpose;
            # X3.T = X1.T + X2.T on DVE.
            x1t = [xtp.tile([128, BL], BF16, tag=f"x1t_{k}", name=f"x1t_{k}") for k in range(KT)]
            x2t = [xtp.tile([128, BL], BF16, tag=f"x2t_{k}", name=f"x2t_{k}") for k in range(KT)]
            x3t = [xtp.tile([128, BL], BF16, tag=f"x3t_{k}", name=f"x3t_{k}") for k in range(KT)]
            def emit_transposes(xtsrc, dst):
                # whole X.T pre-packed on host: straight k-tile DMA loads
                dmas = []
                for k in range(KT):
                    dmas.append(nc.gpsimd.dma_start(out=dst[k][:], in_=xtsrc[k]))
                return dmas

            xts = (x1t, x2t, x3t)

            def cmul(outr, outi, ar, ai, br, bi, pfx):
                """(outr + i*outi) = (ar + i*ai) * (br + i*bi), fp32 DVE."""
                t1 = prodp.tile([128, 128], F32, tag=f"{pfx}1", name=f"{pfx}1")
                t2 = prodp.tile([128, 128], F32, tag=f"{pfx}2", name=f"{pfx}2")
                t3 = prodp.tile([128, 128], F32, tag=f"{pfx}3", name=f"{pfx}3")
                t4 = prodp.tile([128, 128], F32, tag=f"{pfx}4", name=f"{pfx}4")
                nc.vector.tensor_mul(t1[:], ar, br)
                nc.vector.tensor_mul(t2[:], ai, bi)
                nc.vector.tensor_mul(t3[:], ar, bi)
                nc.vector.tensor_mul(t4[:], ai, br)
                nc.vector.tensor_sub(outr, t1[:], t2[:])
                nc.vector.tensor_add(outi, t3[:], t4[:])

            p1s_all = {ob: [None] * MT for ob in range(OB)}
            p2s_all = {ob: [None] * MT for ob in range(OB)}
            pa = [None] * MT

            def emit_mat(ob, mat):
                ocols = slice(ob * 128, (ob + 1) * 128)
                oicols = slice(H + ob * 128, H + (ob + 1) * 128)
                p1s = p1s_all[ob]
                p2s = p2s_all[ob]
                if True:
                    w = wp.tile([128, KT * NW], BF16, tag="wslab", name="wslab")
                    wsrc = wpk[mat, ob].rearrange("p kt c -> p (kt c)")
                    if ob == 0 and mat == 0:
                        # split the very first slab so matmuls start as
                        # quarters land
                        qr = KT * NW // 4
                        for _q in range(4):
                            nc.sync.dma_start(
                                out=w[:, _q * qr : (_q + 1) * qr],
                                in_=wsrc[:, _q * qr : (_q + 1) * qr],
                            )
                    elif ob == 0 and mat == 1:
                        half = KT * NW // 2
                        nc.sync.dma_start(out=w[:, :half], in_=wsrc[:, :half])
                        nc.sync.dma_start(out=w[:, half:], in_=wsrc[:, half:])
                    else:
                        nc.sync.dma_start(out=w[:], in_=wsrc)
                    for m in range(MT):
                        rows = slice(m * 128, (m + 1) * 128)
                        ps = psmm.tile([128, NW], F32, tag="mm", name="mm")
                        for k in range(KT):
                            nc.tensor.matmul(
                                ps[:],
                                lhsT=xts[mat][k][:, rows],
                                rhs=w[:, k * NW : (k + 1) * NW],
                                start=(k == 0),
                                stop=(k == KT - 1),
                            )
                        if mat == 0:
                            p1s[m] = pp.tile([128, NW], F32, tag=f"p1_{m}", name=f"p1_{m}")
                            nc.scalar.copy(p1s[m][:], ps[:])
                        elif mat == 1:
                            p2s[m] = pp.tile([128, NW], F32, tag=f"p2_{m}", name=f"p2_{m}")
                            nc.scalar.copy(p2s[m][:], ps[:])
                            # ---- epilogue phase A: everything that only
                            # needs P1/P2 (not P3) — overlaps the P3 matmuls.
                            obw = slice(ob * NW, (ob + 1) * NW)
                            p1, p2 = p1s[m], p2s[m]
                            zr = epp.tile([128, NW], F32, tag="zra", name="zra")
                            nc.vector.tensor_sub(zr[:], p1[:], p2[:])
                            nc.gpsimd.tensor_add(zr[:], zr[:], bias_r[:, obw])
                            gr = epp.tile([128, NW], F32, tag=f"gr_{m}", name=f"gr_{m}", bufs=1)
                            nc.scalar.activation(gr[:, 0:384], zr[:, 0:384], AFT.Sigmoid)
                            nc.scalar.activation(gr[:, 384:512], zr[:, 384:512], AFT.Tanh)
                            # q = p1 + p2 - bias_i: lets phase B produce
                            # zi = P3 - q in a single DVE op off the bias path
                            q = epp.tile([128, NW], F32, tag=f"q_{m}", name=f"q_{m}", bufs=1)
                            nc.vector.tensor_add(q[:], p1[:], p2[:])
                            nc.gpsimd.tensor_sub(q[:], q[:], bias_i[:, obw])
                            cr = ctile[m][:, ocols]
                            ci = ctile[m][:, oicols]
                            fr = gr[:, 0:128]
                            ir_ = gr[:, 128:256]
                            ar = gr[:, 384:512]
                            u1 = prodp.tile([128, 128], F32, tag=f"u1_{m}", name=f"u1_{m}", bufs=1)
                            u4 = prodp.tile([128, 128], F32, tag=f"u4_{m}", name=f"u4_{m}", bufs=1)
                            v1 = prodp.tile([128, 128], F32, tag=f"v1_{m}", name=f"v1_{m}", bufs=1)
                            nc.vector.tensor_mul(u1[:], cr, fr)
                            nc.vector.tensor_mul(u4[:], ci, fr)
                            nc.vector.tensor_mul(v1[:], ar, ir_)
                            pa[m] = (gr, q, u1, u4, v1)
                        else:
                            # ---- epilogue phase B for (ob, m): ps holds P3 ----
                            obw = slice(ob * NW, (ob + 1) * NW)
                            gr, q, u1, u4, v1 = pa[m]
                            zi = epp.tile([128, NW], F32, tag="zi", name="zi")
                            # halves: the sigmoid can start after the first
                            # half-subtract instead of the full-width op
                            nc.vector.tensor_sub(zi[:, 0:256], ps[:, 0:256], q[:, 0:256])
                            nc.vector.tensor_sub(zi[:, 256:512], ps[:, 256:512], q[:, 256:512])
                            gi = epp.tile([128, NW], F32, tag="gi", name="gi")
                            nc.scalar.activation(gi[:, 0:256], zi[:, 0:256], AFT.Sigmoid)
                            nc.scalar.activation(gi[:, 256:384], zi[:, 256:384], AFT.Sigmoid)
                            nc.scalar.activation(gi[:, 384:512], zi[:, 384:512], AFT.Tanh)
                            cr = ctile[m][:, ocols]
                            ci = ctile[m][:, oicols]
                            fi = gi[:, 0:128]
                            ii_ = gi[:, 128:256]
                            oi = gi[:, 256:384]
                            ai = gi[:, 384:512]
                            ir_ = gr[:, 128:256]
                            orr = gr[:, 256:384]
                            ar = gr[:, 384:512]
                            u2 = prodp.tile([128, 128], F32, tag="u2", name="u2")
                            u3 = prodp.tile([128, 128], F32, tag="u3", name="u3")
                            v2 = prodp.tile([128, 128], F32, tag="v2", name="v2")
                            v3 = prodp.tile([128, 128], F32, tag="v3", name="v3")
                            v4 = prodp.tile([128, 128], F32, tag="v4", name="v4")
                            nc.vector.tensor_mul(u2[:], ci, fi)
                            nc.vector.tensor_mul(u3[:], cr, fi)
                            nc.gpsimd.tensor_mul(v2[:], ai, ii_)
                            nc.gpsimd.tensor_mul(v3[:], ar, ii_)
                            nc.vector.tensor_mul(v4[:], ai, ir_)
                            cfr = prodp.tile([128, 128], F32, tag="cfr", name="cfr")
                            cfi = prodp.tile([128, 128], F32, tag="cfi", name="cfi")
                            air = prodp.tile([128, 128], F32, tag="air", name="air")
                            aii = prodp.tile([128, 128], F32, tag="aii", name="aii")
                            nc.vector.tensor_sub(cfr[:], u1[:], u2[:])
                            nc.vector.tensor_add(cfi[:], u3[:], u4[:])
                            nc.gpsimd.tensor_sub(air[:], v1[:], v2[:])
                            nc.gpsimd.tensor_add(aii[:], v3[:], v4[:])
                            ctr = prodp.tile([128, 128], F32, tag="ctr", name="ctr")
                            cti = prodp.tile([128, 128], F32, tag="cti", name="cti")
                            nc.vector.tensor_add(ctr[:], cfr[:], air[:])
                            nc.vector.tensor_add(cti[:], cfi[:], aii[:])
                            tr = prodp.tile([128, 128], F32, tag="tr", name="tr")
                            ti = prodp.tile([128, 128], F32, tag="ti", name="ti")
                            nc.scalar.activation(tr[:], ctr[:], AFT.Tanh)
                            nc.scalar.activation(ti[:], cti[:], AFT.Tanh)
                            htr = prodp.tile([128, 128], F32, tag="htr", name="htr")
                            hti = prodp.tile([128, 128], F32, tag="hti", name="hti")
                            w1 = prodp.tile([128, 128], F32, tag="w1", name="w1")
                            w2 = prodp.tile([128, 128], F32, tag="w2", name="w2")
                            w3 = prodp.tile([128, 128], F32, tag="w3", name="w3")
                            w4 = prodp.tile([128, 128], F32, tag="w4", name="w4")
                            # real half on DVE, imag half on GPSIMD in parallel
                            nc.vector.tensor_mul(w1[:], orr, tr[:])
                            nc.vector.tensor_mul(w2[:], oi, ti[:])
                            nc.vector.tensor_sub(htr[:], w1[:], w2[:])
                            nc.gpsimd.tensor_mul(w3[:], orr, ti[:])
                            nc.gpsimd.tensor_mul(w4[:], oi, tr[:])
                            nc.gpsimd.tensor_add(hti[:], w3[:], w4[:])
                            nc.sync.dma_start(out=h_out[rows, ocols], in_=htr[:])
                            nc.scalar.dma_start(out=h_out[rows, oicols], in_=hti[:])
                            nc.scalar.dma_start(out=c_out[rows, ocols], in_=ctr[:])
                            nc.sync.dma_start(out=c_out[rows, oicols], in_=cti[:])


            # PE stream order: x1 transposes, then the first matmul block
            # (only needs X1.T), then x2 transposes + x3 adds while that
            # block runs, then everything else.
            d1 = emit_transposes(x1tp, x1t)
            if repeat > 0:
                emit_mat(0, 0)
            d2 = emit_transposes(x2tp, x2t)
            # host-broadcast bias tiles ride the SWDGE ring after all
            # x/h loads (first use is the phase-A gpsimd adds at ~t=30us)
            for _bt, _bi in ((bias_r, 0), (bias_i, 1)):
                d = nc.gpsimd.dma_start(out=_bt[:], in_=bbc[_bi])
                add_dep_helper(d.ins, d2[-1].ins, sync=False,
                               reason="bias loads after x/h loads")
            # c tiles ride the SWDGE ring after all x/h loads (first use of
            # c is the phase-A products at ~t=30us)
            for m in range(MT):
                t = cresp.tile([128, 2 * H], BF16, tag=f"c_m{m}", name=f"c_m{m}")
                d = nc.gpsimd.dma_start(out=t[:], in_=cx[m * 128 : (m + 1) * 128, :])
                add_dep_helper(d.ins, d2[-1].ins, sync=False,
                               reason="c loads after x/h loads")
                ctile.append(t)
            for k in range(KT):
                nc.vector.tensor_add(x3t[k][:], x1t[k][:], x2t[k][:])
            if repeat > 0:
                emit_mat(0, 1)
                emit_mat(0, 2)
                for ob in range(1, OB):
                    for mat in range(3):
                        emit_mat(ob, mat)
            for _rep in range(1, repeat):
                for ob in range(OB):
                    for mat in range(3):
                        emit_mat(ob, mat)
            if sink is not None:
                nc.gpsimd.dma_start(out=sink[:], in_=bias_r[0:1, 0:4])
    return nc


_NC_CACHE = None


def _get_program():
    global _NC_CACHE
    if _NC_CACHE is None:
        nc = _build_program()
        fixed = _split_multiwait_json(nc.to_json_bytes())
        nc.to_json_bytes = lambda: fixed
        _NC_CACHE = nc
    return _NC_CACHE


def _pack_weights(Uw_r, Uw_i, Ww_r, Ww_i, Ub_r, Ub_i, Wb_r, Wb_i):
    GORD = [0, 1, 3, 2]  # gate order f, i, o, a within each o-block:
    # the three sigmoid gates are contiguous so the epilogue needs one
    # sigmoid call [0:384] and one tanh call [384:512] per z tile.

    def interleave_cols(Wg):  # [2048, G, H] -> [2048, GH], c = ob*512+gidx*128+oi
        return (
            Wg.reshape(K, G, OB, 128)[:, GORD]
            .transpose(0, 2, 1, 3)
            .reshape(K, G * H)
        )

    # [G, H(o), D(k)] -> [k, G, o], stack x-side over h-side along k
    Wr = np.concatenate(
        [np.transpose(Uw_r, (2, 0, 1)), np.transpose(Ww_r, (2, 0, 1))], axis=0
    )
    Wi = np.concatenate(
        [np.transpose(Uw_i, (2, 0, 1)), np.transpose(Ww_i, (2, 0, 1))], axis=0
    )
    W1 = interleave_cols(Wr)
    W2 = interleave_cols(Wi)
    W3 = W1 + W2
    Wall = np.stack([W1, W2, W3])  # [3, 2048, 4096] fp32
    # -> [3, ob, p(128), kt, c(512)]: per (mat, ob) this is exactly the
    # SBUF slab layout [128 partitions x (kt*512) free], so the weight
    # DMA is one contiguous 2 MB copy.
    wpk = (
        Wall.reshape(3, KT, 128, OB, NW)
        .transpose(0, 3, 2, 1, 4)
        .astype(ml_dtypes.bfloat16)
    )

    def interleave_bias(b):  # [G, H] -> [GH] interleaved
        return b.reshape(G, OB, 128)[GORD].transpose(1, 0, 2).reshape(G * H)

    br = interleave_bias(Ub_r + Wb_r)
    bi = interleave_bias(Ub_i + Wb_i)
    bbc = np.ascontiguousarray(np.broadcast_to(
        np.stack([br, bi])[:, None, :], (2, 128, G * H)
    ).astype(ml_dtypes.bfloat16))
    return np.ascontiguousarray(wpk), np.ascontiguousarray(bbc)


def kernel(input, h_x, c_x, Uw_r, Uw_i, Ub_r, Ub_i, Ww_r, Ww_i, Wb_r, Wb_i,
           _trace=False):
    input = np.asarray(input, dtype=np.float32)
    h_x = np.asarray(h_x, dtype=np.float32)
    c_x = np.asarray(c_x, dtype=np.float32)
    wpk, bpk = _pack_weights(
        np.asarray(Uw_r, np.float32), np.asarray(Uw_i, np.float32),
        np.asarray(Ww_r, np.float32), np.asarray(Ww_i, np.float32),
        np.asarray(Ub_r, np.float32), np.asarray(Ub_i, np.float32),
        np.asarray(Wb_r, np.float32), np.asarray(Wb_i, np.float32),
    )

    x1b = np.concatenate([input[:, :IN], h_x[:, :H]], axis=1).astype(ml_dtypes.bfloat16)
    x2b = np.concatenate([input[:, IN:], h_x[:, H:]], axis=1).astype(ml_dtypes.bfloat16)

    in_maps = []
    for c in range(NCORES):
        rows = slice(c * BL, (c + 1) * BL)
        in_maps.append(
            {
                "x1tp": np.ascontiguousarray(
                    x1b[rows].T.reshape(KT, 128, BL)
                ),
                "x2tp": np.ascontiguousarray(
                    x2b[rows].T.reshape(KT, 128, BL)
                ),
                "cx": np.ascontiguousarray(c_x[rows].astype(ml_dtypes.bfloat16)),
                "wpk": wpk,
                "bbc": bpk,
            }
        )

    nc = _get_program()
    res = run_bass_kernel_spmd(
        nc, in_maps, core_ids=list(range(NCORES)), trace=_trace
    )
    h_t = np.concatenate([res.results[i]["h_out"] for i in range(NCORES)], axis=0)
    c_t = np.concatenate([res.results[i]["c_out"] for i in range(NCORES)], axis=0)
    if _trace:
        kernel._last_results = res
    return h_t, c_t



# revision 9
# speedup vs baseline: 1.4580x; 1.4580x over previous
"""Complex LSTM cell (CLSTMCell) Trainium2 kernel — fp8 DoubleRow edition.

Full inputs in, full outputs out. Data-parallel over batch: B=4096 rows
sharded 512/core across 8 NeuronCores; weights replicated (host pre-packed).

Math: with X1=[xr|hr], X2=[xi|hi] ([B,2048]) and W1=[Ur;Wr], W2=[Ui;Wi]
([2048,4096]), gate projections run on the PE in fp8-e4m3 DoubleRow mode
(2 contraction rows per cycle). Each fp32 operand A is represented as
fp8 pair A = Ahi + Alo (Alo = fp8 residual, exact in e4m3's wide exponent
range), recovering ~bf16 accuracy from fp8 matmuls at 3 chains/product:
  A@B ~= Ahi@Bhi + Alo@Bhi + Ahi@Blo.

Gates f, a, o use complex Karatsuba (P1=X1W1, P2=X2W2, P3=X3W3 with
X3=X1+X2, W3=W1+W2; Zr=P1-P2, Zi=P3-P1-P2) with fully split products.
Gate i (least error-sensitive) uses the direct form with single-chain
(hi-only) products PSUM-accumulated via a negated X2 copy:
  Zr_i = X1hi@W1hi + (-X2hi)@W2hi,  Zi_i = X1hi@W2hi + X2hi@W1hi.

Matmuls are oriented weights-stationary: PSUM tiles are [128 o-cols,
512 batch], so the gate bias is a per-partition scalar folded into the
Activation op's scale+bias, and the cell-update epilogue runs in fp16 on
DVE at 2x throughput. Outputs return transposed [2H, B/core] in fp16 and
are unpacked on host.
"""

import sys

for _p in ("/opt/trn_rl_repo",):
    if _p not in sys.path:
        sys.path.insert(0, _p)

import numpy as np
import ml_dtypes

import concourse.bass as bass
import concourse.mybir as mybir
from concourse.bass_utils import run_bass_kernel_spmd
from concourse.tile import TileContext

F32 = mybir.dt.float32
F16 = mybir.dt.float16
FP8 = mybir.dt.float8e4
NPFP8 = ml_dtypes.float8_e4m3  # TRN e4m3: max normal 240
AFT = mybir.ActivationFunctionType
DR = mybir.MatmulPerfMode.DoubleRow

B = 4096
IN = 1024
H = 1024
NCORES = 8
BL = B // NCORES          # 512 batch rows per core (= max moving free)
K = 2 * IN                # 2048 contraction
KT = K // 128             # 16 k-tiles
OB = H // 8               # -
NOB = 8                   # o-blocks of 128
SX, SWT = 16.0, 256.0     # fp8 quantization scales
SINV = 1.0 / (SX * SWT)

# X slab order in dram/SBUF (i-gate consumers first so its chains can
# start before the karatsuba residual slabs land)
XS = ("x1h", "x2nh", "x2h", "x1l", "x2l", "x3h", "x3l")
# W slab order per o-block: direct gate i first (first chains emitted),
# then karatsuba gates f, a, o (6 slabs each).
# Gate column order in the packed weight tensor: f, i, a, o (ref order).
KAR_GATES = (0, 2, 3)     # f, a, o
DIR_GATE = 1              # i
WS = [(DIR_GATE, "1h"), (DIR_GATE, "2h")]
for _g in KAR_GATES:
    for _s in ("1h", "1l", "2h", "2l", "3h", "3l"):
        WS.append((_g, _s))
NSLAB = len(WS)           # 20


def _split_multiwait_json(raw: bytes) -> bytes:
    """The walrus build in this container accepts at most one sem wait
    per instruction; Tile's scheduler packs several. Split the extras
    into preceding wait-only EventSemaphore instructions on the same
    engine (same semantics: the sequencer blocks on each in order)."""
    import orjson

    m = orjson.loads(raw)
    ctr = 0
    for fn in m["functions"]:
        for bb in fn["blocks"]:
            out = []
            for ins in bb["instructions"]:
                si = ins.get("sync_info")
                waits = si.get("on_wait") if si else None
                if waits and len(waits) > 1:
                    for w in waits[:-1]:
                        ctr += 1
                        nop = {
                            "engine": ins["engine"],
                            "ins": [],
                            "outs": [],
                            "name": f"{ins['name']}_sw{ctr}",
                            "opcode": "EventSemaphore",
                            "sync_info": {"on_update": [], "on_wait": [w]},
                        }
                        if "debug" in ins:
                            nop["debug"] = ins["debug"]
                        out.append(nop)
                    si["on_wait"] = [waits[-1]]
                out.append(ins)
            bb["instructions"] = out
    return orjson.dumps(m)


def _build_program():
    nc = bass.Bass()

    xpk = nc.dram_tensor("xpk", [len(XS), 128, KT, BL], FP8, kind="ExternalInput")
    wpk = nc.dram_tensor("wpk", [NOB, NSLAB, 128, KT * 128], FP8, kind="ExternalInput")
    ctp = nc.dram_tensor("ctp", [128, 16, BL], F16, kind="ExternalInput")
    bpk = nc.dram_tensor("bpk", [128, 64], F32, kind="ExternalInput")
    ht_out = nc.dram_tensor("ht_out", [16, 128, BL], F16, kind="ExternalOutput")
    ct_out = nc.dram_tensor("ct_out", [16, 128, BL], F16, kind="ExternalOutput")

    with TileContext(nc) as tc:
        with (
            tc.tile_pool(name="xc", bufs=1) as xcp,
            tc.tile_pool(name="w", bufs=2) as wp,
            tc.tile_pool(name="z", bufs=2) as zp,
            tc.tile_pool(name="g", bufs=2) as gp,
            tc.tile_pool(name="cell", bufs=1) as cp,
            tc.tile_pool(name="ps", bufs=8, space="PSUM") as pp,
        ):
            # resident inputs
            xt = {}
            for si, sname in enumerate(XS):
                t = xcp.tile([128, KT, BL], FP8, tag=f"x_{sname}", name=f"x_{sname}")
                nc.sync.dma_start(out=t[:], in_=xpk[si])
                xt[sname] = t
            ct = xcp.tile([128, 16, BL], F16, tag="ct", name="ct")
            nc.gpsimd.dma_start(out=ct[:], in_=ctp[:])
            bias = xcp.tile([128, 64], F32, tag="bias", name="bias")
            nc.gpsimd.dma_start(out=bias[:], in_=bpk[:])

            def chain_mm(ps, wslab, xnames, first, last):
                # accumulate sum_k W[k].T @ X[k] (DoubleRow k-pairs) for one
                # or more (wslab, xslab) products into psum tile ps
                n = len(xnames)
                for ci, xn in enumerate(xnames):
                    xs = xt[xn]
                    for kp in range(KT // 2):
                        nc.tensor.matmul(
                            ps[:],
                            lhsT=wslab[ci][:, 2 * kp : 2 * kp + 2, :],
                            rhs=xs[:, 2 * kp : 2 * kp + 2, :],
                            start=(first and ci == 0 and kp == 0),
                            stop=(last and ci == n - 1 and kp == KT // 2 - 1),
                            perf_mode=DR,
                        )

            for ob in range(NOB):
                # weight slabs for this o-block (one DMA per slab kind)
                ws = {}
                for si, (g, sname) in enumerate(WS):
                    t = wp.tile(
                        [128, KT, 128], FP8, tag=f"w{si}", name=f"w_{g}_{sname}"
                    )
                    nc.scalar.dma_start(
                        out=t[:],
                        in_=wpk[ob, si].rearrange("p (kt o) -> p kt o", kt=KT),
                    )
                    ws[(g, sname)] = t

                gates = {}  # (gate, 'r'/'i') -> fp16 [128, BL]

                def act_gate(g, part, src, func, scale=SINV):
                    bcol = g * 16 + ob * 2 + (0 if part == "r" else 1)
                    out = gp.tile(
                        [128, BL], F16, tag=f"g{g}{part}", name=f"g{g}{part}_{ob}"
                    )
                    nc.scalar.activation(
                        out[:], src[:], func,
                        bias=bias[:, bcol : bcol + 1], scale=scale,
                    )
                    gates[(g, part)] = out

                # ---- gate i (direct, hi-only, PSUM-accumulated) ----
                zri = pp.tile([128, BL], F32, tag="ps", name=f"zri_{ob}")
                chain_mm(zri, [ws[(1, "1h")], ws[(1, "2h")]], ["x1h", "x2nh"],
                         True, True)
                act_gate(1, "r", zri, AFT.Sigmoid)
                zii = pp.tile([128, BL], F32, tag="ps", name=f"zii_{ob}")
                chain_mm(zii, [ws[(1, "2h")], ws[(1, "1h")]], ["x1h", "x2h"],
                         True, True)
                act_gate(1, "i", zii, AFT.Sigmoid)

                # ---- karatsuba gates f, a, o ----
                for g in KAR_GATES:
                    func = AFT.Tanh if g == 2 else AFT.Sigmoid
                    P = []
                    for p, xb in ((1, "x1"), (2, "x2"), (3, "x3")):
                        ps = pp.tile([128, BL], F32, tag="ps", name=f"p{p}_{g}_{ob}")
                        chain_mm(
                            ps,
                            [ws[(g, f"{p}h")], ws[(g, f"{p}h")], ws[(g, f"{p}l")]],
                            [f"{xb}h", f"{xb}l", f"{xb}h"],
                            True, True,
                        )
                        P.append(ps)
                    p1, p2, p3 = P
                    # DVE may read only one PSUM operand per op: stage P2
                    # into SBUF via the Activation engine first.
                    p2s = zp.tile([128, BL], F32, tag="p2s", name=f"p2s_{g}_{ob}")
                    nc.scalar.copy(p2s[:], p2[:])
                    zr = zp.tile([128, BL], F32, tag="zr", name=f"zr_{g}_{ob}")
                    q = zp.tile([128, BL], F32, tag="q", name=f"q_{g}_{ob}")
                    zi = zp.tile([128, BL], F32, tag="zi", name=f"zi_{g}_{ob}")
                    nc.vector.tensor_sub(zr[:], p1[:], p2s[:])
                    nc.vector.tensor_add(q[:], p1[:], p2s[:])
                    nc.vector.tensor_sub(zi[:], p3[:], q[:])
                    act_gate(g, "r", zr, func)
                    act_gate(g, "i", zi, func)

                # ---- cell update (fp16 on DVE) ----
                cr = ct[:, ob, :]
                ci = ct[:, 8 + ob, :]

                def tmp(tag):
                    return cp.tile([128, BL], F16, tag=tag, name=f"{tag}_{ob}")

                fr, fi = gates[(0, "r")], gates[(0, "i")]
                ir_, ii_ = gates[(1, "r")], gates[(1, "i")]
                ar, ai = gates[(2, "r")], gates[(2, "i")]
                orr, oi = gates[(3, "r")], gates[(3, "i")]

                u1, u2, u3, u4 = tmp("u1"), tmp("u2"), tmp("u3"), tmp("u4")
                v1, v2, v3, v4 = tmp("v1"), tmp("v2"), tmp("v3"), tmp("v4")
                nc.vector.tensor_mul(u1[:], cr, fr[:])
                nc.vector.tensor_mul(u2[:], ci, fi[:])
                nc.vector.tensor_mul(u3[:], cr, fi[:])
                nc.vector.tensor_mul(u4[:], ci, fr[:])
                nc.vector.tensor_mul(v1[:], ar[:], ir_[:])
                nc.vector.tensor_mul(v2[:], ai[:], ii_[:])
                nc.vector.tensor_mul(v3[:], ar[:], ii_[:])
                nc.vector.tensor_mul(v4[:], ai[:], ir_[:])
                cfr, cfi = tmp("cfr"), tmp("cfi")
                air, aii = tmp("air"), tmp("aii")
                nc.vector.tensor_sub(cfr[:], u1[:], u2[:])
                nc.vector.tensor_add(cfi[:], u3[:], u4[:])
                nc.vector.tensor_sub(air[:], v1[:], v2[:])
                nc.vector.tensor_add(aii[:], v3[:], v4[:])
                ctr, cti = tmp("ctr"), tmp("cti")
                nc.vector.tensor_add(ctr[:], cfr[:], air[:])
                nc.vector.tensor_add(cti[:], cfi[:], aii[:])
                nc.gpsimd.dma_start(out=ct_out[ob], in_=ctr[:])
                nc.gpsimd.dma_start(out=ct_out[8 + ob], in_=cti[:])
                trr, tri = tmp("trr"), tmp("tri")
                nc.scalar.activation(trr[:], ctr[:], AFT.Tanh)
                nc.scalar.activation(tri[:], cti[:], AFT.Tanh)
                w1, w2, w3, w4 = tmp("w1"), tmp("w2"), tmp("w3"), tmp("w4")
                nc.vector.tensor_mul(w1[:], orr[:], trr[:])
                nc.vector.tensor_mul(w2[:], oi[:], tri[:])
                nc.vector.tensor_mul(w3[:], orr[:], tri[:])
                nc.vector.tensor_mul(w4[:], oi[:], trr[:])
                htr, hti = tmp("htr"), tmp("hti")
                nc.vector.tensor_sub(htr[:], w1[:], w2[:])
                nc.vector.tensor_add(hti[:], w3[:], w4[:])
                nc.sync.dma_start(out=ht_out[ob], in_=htr[:])
                nc.sync.dma_start(out=ht_out[8 + ob], in_=hti[:])
    return nc


_NC_CACHE = None


def _get_program():
    global _NC_CACHE
    if _NC_CACHE is None:
        nc = _build_program()
        fixed = _split_multiwait_json(nc.to_json_bytes())
        nc.to_json_bytes = lambda: fixed
        _NC_CACHE = nc
    return _NC_CACHE


def _q8(a):
    return np.clip(a, -240.0, 240.0).astype(NPFP8)


def _split8(a):
    hi = _q8(a)
    lo = _q8(a - hi.astype(np.float32))
    return hi, lo


def _pack_x(x1, x2):
    # x1, x2: [BL, 2048] fp32 (pre-scaled). returns [7, 128, KT, BL] fp8
    out = np.empty((len(XS), 128, KT, BL), NPFP8)
    x3 = x1 + x2
    h1, l1 = _split8(x1)
    h2, l2 = _split8(x2)
    h3, l3 = _split8(x3)
    n2 = (-h2.astype(np.float32)).astype(NPFP8)
    for si, arr in enumerate((h1, n2, h2, l1, l2, h3, l3)):
        # [BL, K] -> X.T k-tiles [128, KT, BL]
        out[si] = arr.T.reshape(KT, 128, BL).transpose(1, 0, 2)
    return out


def _pack_w(W1, W2):
    # W1, W2: [2048, 4H] fp32 (pre-scaled), gate-major columns [f,i,a,o].
    # returns [NOB, NSLAB, 128, KT*128] fp8
    W3 = W1 + W2
    spl = {}
    for p, W in ((1, W1), (2, W2), (3, W3)):
        spl[f"{p}h"], spl[f"{p}l"] = _split8(W)
    out = np.empty((NOB, NSLAB, 128, KT * 128), NPFP8)
    for si, (g, sname) in enumerate(WS):
        Wg = spl[sname][:, g * H : (g + 1) * H]  # [2048, 1024]
        for ob in range(NOB):
            blk = Wg[:, ob * 128 : (ob + 1) * 128]  # [2048, 128]
            out[ob, si] = (
                blk.reshape(KT, 128, 128).transpose(1, 0, 2).reshape(128, KT * 128)
            )
    return out


def kernel(input, h_x, c_x, Uw_r, Uw_i, Ub_r, Ub_i, Ww_r, Ww_i, Wb_r, Wb_i,
           _trace=False):
    input = np.asarray(input, dtype=np.float32)
    h_x = np.asarray(h_x, dtype=np.float32)
    c_x = np.asarray(c_x, dtype=np.float32)

    W1 = np.concatenate(
        [np.transpose(np.asarray(Uw_r, np.float32), (2, 0, 1)),
         np.transpose(np.asarray(Ww_r, np.float32), (2, 0, 1))], axis=0
    ).reshape(K, 4 * H) * SWT
    W2 = np.concatenate(
        [np.transpose(np.asarray(Uw_i, np.float32), (2, 0, 1)),
         np.transpose(np.asarray(Ww_i, np.float32), (2, 0, 1))], axis=0
    ).reshape(K, 4 * H) * SWT
    wpk = _pack_w(W1, W2)

    br = (np.asarray(Ub_r, np.float32) + np.asarray(Wb_r, np.float32)).reshape(4 * H)
    bi = (np.asarray(Ub_i, np.float32) + np.asarray(Wb_i, np.float32)).reshape(4 * H)
    # bias tile [128, 64]: col = gate*16 + ob*2 + (0 r | 1 i)
    bpk = np.empty((128, 64), np.float32)
    for g in range(4):
        for ob in range(NOB):
            bpk[:, g * 16 + ob * 2 + 0] = br[g * H + ob * 128 : g * H + (ob + 1) * 128]
            bpk[:, g * 16 + ob * 2 + 1] = bi[g * H + ob * 128 : g * H + (ob + 1) * 128]

    X1 = np.concatenate([input[:, :IN], h_x[:, :H]], axis=1) * SX
    X2 = np.concatenate([input[:, IN:], h_x[:, H:]], axis=1) * SX

    in_maps = []
    for c in range(NCORES):
        rows = slice(c * BL, (c + 1) * BL)
        # c_x transposed: [2H, BL] -> [128, 16, BL] (block j = rows j*128+)
        cT = np.ascontiguousarray(
            c_x[rows].T.reshape(16, 128, BL).transpose(1, 0, 2)
        ).astype(np.float16)
        in_maps.append(
            {
                "xpk": _pack_x(X1[rows], X2[rows]),
                "wpk": wpk,
                "ctp": cT,
                "bpk": bpk,
            }
        )

    nc = _get_program()
    res = run_bass_kernel_spmd(
        nc, in_maps, core_ids=list(range(NCORES)), trace=_trace
    )
    h_parts, c_parts = [], []
    for i in range(NCORES):
        hT = res.results[i]["ht_out"].reshape(2 * H, BL)
        cT = res.results[i]["ct_out"].reshape(2 * H, BL)
        h_parts.append(hT.T)
        c_parts.append(cT.T)
    h_t = np.concatenate(h_parts, axis=0).astype(np.float32)
    c_t = np.concatenate(c_parts, axis=0).astype(np.float32)
    if _trace:
        kernel._last_results = res
    return h_t, c_t
